# revision 1
# baseline (speedup 1.0000x reference)
"""CTRNN kernel for Trainium2 (Bass/Tile), data-parallel over batch on 8 cores.

Math (per reference):
    u_t = x_t @ W_in.T + b_in                        # input projection
    h_t = 0.9 * h_{t-1} + 0.1 * relu(u_t + h_{t-1} @ W_hh.T + b_hh)
    output = stack(h_t for t in 1..T), hidden = h_T

Sharding: batch 256 -> 32 per core; weights replicated; the sequential
time loop runs locally per core.

Device-side design (per core, B=32, H=1024, I=256, T=512):
  - State is kept transposed+packed in SBUF: hT[p, 32*sigma + b] =
    h[b, 128*k(sigma) + p], k(sigma) = 2*(sigma%4) + sigma//4.
  - Each step accumulates bias + W_in*x_t + W_hh*h into one PSUM tile
    [128, 256] laid out as y[32g+b, c] = (pre-activation)[b, 256g+c],
    using 4 concurrently-streaming col-tiled (tile_position) f32r
    matmuls per contraction chunk.
  - ScalarE computes 0.1*relu from PSUM; two PE transpose-matmuls
    restore the transposed layout; one fused VectorE op applies the
    leaky blend hT_new = 0.9*hT + rT.
  - x_t is transposed on the PE in blocks of 4 timesteps.
Host side pre-transposes the weight matrices and re-assembles/un-packs
the per-step transposed outputs.
"""

from contextlib import ExitStack

import numpy as np

import concourse.bacc as bacc
import concourse.mybir as mybir
import concourse.tile as tile
from concourse.bass_utils import run_bass_kernel_spmd

F32 = mybir.dt.float32
F32R = mybir.dt.float32r
ALPHA = 0.1
T, B, I, H = 512, 256, 256, 1024
NCORES = 8
BS = B // NCORES  # 32
K_OF_SIGMA = [2 * (s % 4) + s // 4 for s in range(8)]  # [0,2,4,6,1,3,5,7]

_CACHE = {}


def _build(tc, out_ap, x_ap, whhT_ap, winT_ap, bias_ap, sel4_ap, ident_ap):
    nc = tc.nc
    ctx = ExitStack()
    const_pool = ctx.enter_context(tc.tile_pool(name="const", bufs=1))
    state_pool = ctx.enter_context(tc.tile_pool(name="state", bufs=2))
    x_pool = ctx.enter_context(tc.tile_pool(name="x", bufs=3))
    xt_pool = ctx.enter_context(tc.tile_pool(name="xt", bufs=2))
    r_pool = ctx.enter_context(tc.tile_pool(name="r", bufs=3))
    ypsum_pool = ctx.enter_context(tc.tile_pool(name="ypsum", bufs=2, space="PSUM"))
    tpsum_pool = ctx.enter_context(tc.tile_pool(name="tpsum", bufs=2, space="PSUM"))
    xtpsum_pool = ctx.enter_context(tc.tile_pool(name="xtpsum", bufs=2, space="PSUM"))

    whh_sb = const_pool.tile([128, 8192], F32, tag="whh")
    for k in range(8):
        nc.sync.dma_start(whh_sb[:, 1024 * k:1024 * (k + 1)],
                          whhT_ap[128 * k:128 * (k + 1), :])
    win_sb = const_pool.tile([128, 2048], F32, tag="win")
    for i in range(2):
        nc.sync.dma_start(win_sb[:, 1024 * i:1024 * (i + 1)],
                          winT_ap[128 * i:128 * (i + 1), :])
    bias_sb = const_pool.tile([4, 256], F32, tag="bias")
    nc.sync.dma_start(bias_sb[:], bias_ap[:])
    sel4_sb = const_pool.tile([4, 128], F32, tag="sel4")
    nc.sync.dma_start(sel4_sb[:], sel4_ap[:])
    ident_sb = const_pool.tile([128, 128], F32, tag="ident")
    nc.sync.dma_start(ident_sb[:], ident_ap[:])
    zeros_sb = const_pool.tile([4, 256], F32, tag="zeros")
    nc.vector.memset(zeros_sb[:], 0.0)

    hT = state_pool.tile([128, 256], F32, tag="hT")
    nc.vector.memset(hT[:], 0.0)

    xt4_sb = None
    for t in range(T):
        tq, tr = divmod(t, 4)
        if tr == 0:
            x4 = x_pool.tile([128, 256], F32, tag="x4")
            nc.sync.dma_start(x4[:], x_ap[4 * tq:4 * tq + 4].rearrange(
                "t b i -> (t b) i"))
            xtp = xtpsum_pool.tile([128, 256], F32, tag="xtp")
            for m in range(2):
                nc.tensor.transpose(xtp[:, 128 * m:128 * (m + 1)],
                                    x4[:, 128 * m:128 * (m + 1)], ident_sb[:])
            xt4_sb = xt_pool.tile([128, 256], F32, tag="xt4")
            nc.vector.tensor_copy(xt4_sb[:], xtp[:])

        ypsum = ypsum_pool.tile([128, 256], F32, tag="ypsum")
        # bias: single full-width K=4 selector matmul opens the psum group
        nc.tensor.matmul(
            ypsum[:], sel4_sb[:].bitcast(F32R), bias_sb[:].bitcast(F32R),
            start=True, stop=False)
        for m in range(2):
            lhs = xt4_sb[:, 128 * m + 32 * tr:128 * m + 32 * tr + 32]
            for g in range(4):
                nc.tensor.matmul(
                    ypsum[32 * g:32 * (g + 1), :],
                    lhs.bitcast(F32R),
                    win_sb[:, 1024 * m + 256 * g:1024 * m + 256 * (g + 1)].bitcast(F32R),
                    start=False, stop=False, tile_position=(0, 32 * g))
        for sigma in range(8):
            k = K_OF_SIGMA[sigma]
            for g in range(4):
                nc.tensor.matmul(
                    ypsum[32 * g:32 * (g + 1), :],
                    hT[:, 32 * sigma:32 * (sigma + 1)].bitcast(F32R),
                    whh_sb[:, 1024 * k + 256 * g:1024 * k + 256 * (g + 1)].bitcast(F32R),
                    start=False, stop=False, tile_position=(0, 32 * g))
        # full-width zero matmul closes the accumulation group (sim tracks
        # psum groups at whole-tile granularity; adds 0 on hardware)
        nc.tensor.matmul(
            ypsum[:], sel4_sb[:].bitcast(F32R), zeros_sb[:].bitcast(F32R),
            start=False, stop=True)

        r_sb = r_pool.tile([128, 256], F32, tag="r")
        nc.scalar.activation(r_sb[:], ypsum[:],
                             mybir.ActivationFunctionType.Relu, scale=ALPHA)

        tpsum = tpsum_pool.tile([128, 256], F32, tag="tpsum")
        for m in range(2):
            nc.tensor.transpose(tpsum[:, 128 * m:128 * (m + 1)],
                                r_sb[:, 128 * m:128 * (m + 1)], ident_sb[:])

        hT_new = state_pool.tile([128, 256], F32, tag="hT")
        nc.vector.scalar_tensor_tensor(
            hT_new[:], hT[:], 1.0 - ALPHA, tpsum[:],
            mybir.AluOpType.mult, mybir.AluOpType.add)
        hT = hT_new

        nc.sync.dma_start(out_ap[t], hT[:])

    ctx.close()


def _get_compiled():
    if "nc" in _CACHE:
        return _CACHE["nc"]
    nc = bacc.Bacc("TRN2", target_bir_lowering=False, debug=False,
                   num_devices=NCORES)
    aps = {}
    for name, shape in [("x", [T, BS, I]), ("whhT", [H, H]), ("winT", [I, H]),
                        ("bias", [4, 256]), ("sel4", [4, 128]),
                        ("ident", [128, 128])]:
        aps[name] = nc.dram_tensor(name, shape, F32, kind="ExternalInput").ap()
    out_ap = nc.dram_tensor("out", [T, 128, 256], F32,
                            kind="ExternalOutput").ap()
    with tile.TileContext(nc) as tc:
        _build(tc, out_ap, aps["x"], aps["whhT"], aps["winT"], aps["bias"],
               aps["sel4"], aps["ident"])
    nc.compile()
    _CACHE["nc"] = nc
    return nc


def _decode(arr):
    """[T, 128, 256] packed-transposed -> [T, 32, 1024]."""
    t = arr.shape[0]
    tmp = arr.reshape(t, 128, 8, 32).transpose(0, 3, 2, 1)  # [t, b, sigma, p]
    sigma_of_k = [4 * (k % 2) + k // 2 for k in range(8)]
    return tmp[:, :, sigma_of_k, :].reshape(t, 32, 1024)


def run_on_hw(x, W_in, b_in, W_hh, b_hh, trace=False, **spmd_kwargs):
    nc = _get_compiled()
    x = np.ascontiguousarray(np.asarray(x, dtype=np.float32))
    whhT = np.ascontiguousarray(np.asarray(W_hh, np.float32).T)
    winT = np.ascontiguousarray(np.asarray(W_in, np.float32).T)
    bias = np.ascontiguousarray(
        (np.asarray(b_in, np.float32) + np.asarray(b_hh, np.float32))
        .reshape(4, 256))
    sel4 = np.repeat(np.eye(4, dtype=np.float32), 32, axis=1)
    ident = np.eye(128, dtype=np.float32)
    in_maps = []
    for c in range(NCORES):
        in_maps.append({
            "x": np.ascontiguousarray(x[:, c * BS:(c + 1) * BS, :]),
            "whhT": whhT, "winT": winT, "bias": bias, "sel4": sel4,
            "ident": ident,
        })
    res = run_bass_kernel_spmd(nc, in_maps, list(range(NCORES)), trace=trace,
                               **spmd_kwargs)
    shards = [_decode(res.results[c]["out"]) for c in range(NCORES)]
    output = np.concatenate(shards, axis=1)  # [T, 256, 1024]
    return output, res


def kernel(x, W_in, b_in, W_hh, b_hh):
    output, _ = run_on_hw(x, W_in, b_in, W_hh, b_hh, trace=False)
    hidden = output[-1].copy()
    return output, hidden


# revision 2
# speedup vs baseline: 1.3039x; 1.3039x over previous
"""CTRNN kernel for Trainium2 (Bass/Tile), data-parallel over batch on 8 cores.

Reference math:
    u_t = x_t @ W_in.T + b_in
    h_t = 0.9 * h_{t-1} + 0.1 * relu(u_t + h_{t-1} @ W_hh.T + b_hh)
    output = stack(h_1..h_T) [T, B, H], hidden = h_T

Sharding: batch 256 -> 32 per core; weights replicated; the sequential time
loop runs locally per core (no collectives).

Device-side design (per core, B=32, H=1024, I=256, T=512):
  - The hidden state lives in a transposed, 32x32-block-permuted SBUF
    layout hT[p, 32w + b] = h[b, Hidx(p, w)], Hidx(p, w) =
    256*(p//32) + 32*w + (p%32). This makes the per-step transposition an
    in-place VectorE 32x32 stream-transpose, with the matching row
    permutation folded into a host-side pre-permuted W_hh^T.
  - Each step accumulates bias + W_in*x_t + W_hh*h into one PSUM tile
    [128, 256] (packed y[32g+b, c] = (preact)[b, 256g+c]) via col-tiled
    (tile_position) bf16 matmuls: 4 concurrent 32-batch-column streams
    per 128-contraction pass, 11 passes per step (1 bias + 2 input + 8
    recurrent), PSUM accumulating in fp32.
  - Epilogue per 128-col half: relu(0.1*y) (VectorE half / ScalarE half),
    VectorE stream-transpose, fused leaky-blend scalar_tensor_tensor into
    bf16 (next matmul operand) and fp32 (accumulator state, DMA'd out).
  - Front-matter matmuls of step t+1 are emitted before the epilogue of
    step t so the TensorE stays busy during the epilogue chain.
Host side pre-permutes/transposes the weights and decodes the per-step
block-transposed outputs.
"""

from contextlib import ExitStack

import numpy as np

import concourse.bacc as bacc
import concourse.mybir as mybir
import concourse.tile as tile
from concourse.bass_utils import run_bass_kernel_spmd

F32 = mybir.dt.float32
BF16 = mybir.dt.bfloat16
ALPHA = 0.1
T, B, I, H = 512, 256, 256, 1024
NCORES = 8
BS = B // NCORES  # 32

_CACHE = {}


def _build(tc, out_ap, x_ap, whhP_ap, winT_ap, bias_ap, sel4_ap, ident_ap):
    nc = tc.nc
    MAX = mybir.AluOpType.max
    MULT = mybir.AluOpType.mult
    ADD = mybir.AluOpType.add

    ctx = ExitStack()
    const_pool = ctx.enter_context(tc.tile_pool(name="const", bufs=1))
    state_pool = ctx.enter_context(tc.tile_pool(name="state", bufs=2))
    x_pool = ctx.enter_context(tc.tile_pool(name="x", bufs=2))
    xt_pool = ctx.enter_context(tc.tile_pool(name="xt", bufs=2))
    r_pool = ctx.enter_context(tc.tile_pool(name="r", bufs=2))
    rt_pool = ctx.enter_context(tc.tile_pool(name="rt", bufs=2))
    ypsum_pool = ctx.enter_context(tc.tile_pool(name="ypsum", bufs=2, space="PSUM"))
    xtpsum_pool = ctx.enter_context(tc.tile_pool(name="xtpsum", bufs=2, space="PSUM"))

    stage = const_pool.tile([128, 8192], F32, tag="stage")
    whh_sb = const_pool.tile([128, 8192], BF16, tag="whh")
    nc.sync.dma_start(stage[:], whhP_ap[:])
    nc.vector.tensor_copy(whh_sb[:], stage[:])
    win_stage = const_pool.tile([128, 2048], F32, tag="win_stage")
    win_sb = const_pool.tile([128, 2048], BF16, tag="win")
    for i in range(2):
        nc.sync.dma_start(win_stage[:, 1024 * i:1024 * (i + 1)],
                          winT_ap[128 * i:128 * (i + 1), :])
    nc.vector.tensor_copy(win_sb[:], win_stage[:])
    bias_stage = const_pool.tile([4, 256], F32, tag="bias_stage")
    bias_sb = const_pool.tile([4, 256], BF16, tag="bias")
    nc.sync.dma_start(bias_stage[:], bias_ap[:])
    nc.vector.tensor_copy(bias_sb[:], bias_stage[:])
    sel4_stage = const_pool.tile([4, 128], F32, tag="sel4_stage")
    sel4_sb = const_pool.tile([4, 128], BF16, tag="sel4")
    nc.sync.dma_start(sel4_stage[:], sel4_ap[:])
    nc.vector.tensor_copy(sel4_sb[:], sel4_stage[:])
    ident_sb = const_pool.tile([128, 128], F32, tag="ident")
    nc.sync.dma_start(ident_sb[:], ident_ap[:])
    zero_f32 = const_pool.tile([128, 256], F32, tag="zero_f32")
    nc.vector.memset(zero_f32[:], 0.0)

    hT = state_pool.tile([128, 256], F32, tag="hT")
    nc.vector.tensor_copy(hT[:], zero_f32[:])
    hT16 = state_pool.tile([128, 256], BF16, tag="hT16")
    nc.vector.tensor_copy(hT16[:], zero_f32[:])

    xt4_sb = None

    def stage_x(q):
        nonlocal xt4_sb
        x4 = x_pool.tile([128, 256], F32, tag="x4")
        nc.sync.dma_start(x4[:], x_ap[4 * q:4 * q + 4].rearrange(
            "t b i -> (t b) i"))
        xtp = xtpsum_pool.tile([128, 256], F32, tag="xtp")
        for m in range(2):
            nc.tensor.transpose(xtp[:, 128 * m:128 * (m + 1)],
                                x4[:, 128 * m:128 * (m + 1)], ident_sb[:])
        xt4_sb = xt_pool.tile([128, 256], BF16, tag="xt4")
        nc.vector.tensor_copy(xt4_sb[:], xtp[:])

    def front_matter(t):
        yp = ypsum_pool.tile([128, 256], F32, tag="ypsum")
        nc.tensor.matmul(yp[:], sel4_sb[:], bias_sb[:],
                         start=True, stop=False, skip_group_check=True)
        tr = t % 4
        for m in range(2):
            lhs = xt4_sb[:, 128 * m + 32 * tr:128 * m + 32 * tr + 32]
            for g in range(4):
                nc.tensor.matmul(
                    yp[32 * g:32 * (g + 1), :], lhs,
                    win_sb[:, 1024 * m + 256 * g:1024 * m + 256 * (g + 1)],
                    start=False, stop=False, tile_position=(0, 32 * g),
                    skip_group_check=True)
        return yp

    stage_x(0)
    ypsum = front_matter(0)

    for t in range(T):
        for w in range(8):
            for g in range(4):
                nc.tensor.matmul(
                    ypsum[32 * g:32 * (g + 1), :],
                    hT16[:, 32 * w:32 * (w + 1)],
                    whh_sb[:, 1024 * w + 256 * g:1024 * w + 256 * (g + 1)],
                    start=False, stop=False, tile_position=(0, 32 * g),
                    skip_group_check=True)

        ypsum_next = None
        if t + 1 < T:
            if (t + 1) % 4 == 0:
                stage_x((t + 1) // 4)
            ypsum_next = front_matter(t + 1)

        r_sb = r_pool.tile([128, 256], F32, tag="r")
        rt_sb = rt_pool.tile([128, 256], F32, tag="rt")
        hT16_new = state_pool.tile([128, 256], BF16, tag="hT16")
        hT_new = state_pool.tile([128, 256], F32, tag="hT")
        h0, h1 = slice(0, 128), slice(128, 256)
        nc.vector.tensor_scalar(r_sb[:, h0], ypsum[:, h0], 0.0, ALPHA, MAX, MULT)
        nc.scalar.activation(r_sb[:, h1], ypsum[:, h1],
                             mybir.ActivationFunctionType.Relu, scale=ALPHA)
        nc.vector.transpose(rt_sb[:, h0], r_sb[:, h0])
        nc.vector.scalar_tensor_tensor(
            hT16_new[:, h0], hT[:, h0], 1.0 - ALPHA, rt_sb[:, h0], MULT, ADD)
        nc.vector.transpose(rt_sb[:, h1], r_sb[:, h1])
        nc.vector.scalar_tensor_tensor(
            hT16_new[:, h1], hT[:, h1], 1.0 - ALPHA, rt_sb[:, h1], MULT, ADD)
        nc.vector.scalar_tensor_tensor(
            hT_new[:], hT[:], 1.0 - ALPHA, rt_sb[:], MULT, ADD)
        nc.sync.dma_start(out_ap[t], hT_new[:])

        hT, hT16, ypsum = hT_new, hT16_new, ypsum_next

    ctx.close()


def _get_compiled():
    if "nc" in _CACHE:
        return _CACHE["nc"]
    nc = bacc.Bacc("TRN2", target_bir_lowering=False, debug=False,
                   num_devices=NCORES)
    aps = {}
    for name, shape in [("x", [T, BS, I]), ("whhP", [128, 8192]),
                        ("winT", [I, H]), ("bias", [4, 256]),
                        ("sel4", [4, 128]), ("ident", [128, 128])]:
        aps[name] = nc.dram_tensor(name, shape, F32, kind="ExternalInput").ap()
    out_ap = nc.dram_tensor("out", [T, 128, 256], F32,
                            kind="ExternalOutput").ap()
    with tile.TileContext(nc) as tc:
        _build(tc, out_ap, aps["x"], aps["whhP"], aps["winT"], aps["bias"],
               aps["sel4"], aps["ident"])
    nc.compile()
    _CACHE["nc"] = nc
    return nc


def host_prep(x_full, W_in, b_in, W_hh, b_hh):
    WT = np.ascontiguousarray(np.asarray(W_hh, np.float32).T)
    p = np.arange(128)
    Hpw = 256 * (p[:, None] // 32) + 32 * np.arange(8)[None, :] + (p[:, None] % 32)
    whhP = np.ascontiguousarray(WT[Hpw].reshape(128, 8192))
    winT = np.ascontiguousarray(np.asarray(W_in, np.float32).T)
    bias = np.ascontiguousarray(
        (np.asarray(b_in, np.float32) + np.asarray(b_hh, np.float32))
        .reshape(4, 256))
    sel4 = np.ascontiguousarray(np.repeat(np.eye(4, dtype=np.float32), 32, axis=1))
    ident = np.eye(128, dtype=np.float32)
    x_full = np.asarray(x_full, np.float32)
    in_maps = []
    for c in range(NCORES):
        in_maps.append({
            "x": np.ascontiguousarray(x_full[:, c * BS:(c + 1) * BS, :]),
            "whhP": whhP, "winT": winT, "bias": bias, "sel4": sel4,
            "ident": ident,
        })
    return in_maps


def decode_out(arr):
    """[T, 128, 256] block-transposed -> [T, 32, 1024]."""
    t = arr.shape[0]
    tmp = arr.reshape(t, 4, 32, 8, 32)  # [t, pg, pv, w, b]
    return np.ascontiguousarray(tmp.transpose(0, 4, 1, 3, 2)).reshape(t, 32, 1024)


def run_on_hw(x, W_in, b_in, W_hh, b_hh, trace=False, **spmd_kwargs):
    nc = _get_compiled()
    in_maps = host_prep(x, W_in, b_in, W_hh, b_hh)
    res = run_bass_kernel_spmd(nc, in_maps, list(range(NCORES)), trace=trace,
                               **spmd_kwargs)
    shards = [decode_out(res.results[c]["out"]) for c in range(NCORES)]
    output = np.concatenate(shards, axis=1)  # [T, 256, 1024]
    return output, res


def kernel(x, W_in, b_in, W_hh, b_hh):
    output, _ = run_on_hw(x, W_in, b_in, W_hh, b_hh, trace=False)
    hidden = output[-1].copy()
    return output, hidden


# revision 3
# speedup vs baseline: 1.3551x; 1.0393x over previous
"""CTRNN kernel for Trainium2 (Bass/Tile), data-parallel over batch on 8 cores.

Reference math:
    u_t = x_t @ W_in.T + b_in
    h_t = 0.9 * h_{t-1} + 0.1 * relu(u_t + h_{t-1} @ W_hh.T + b_hh)
    output = stack(h_1..h_T) [T, B, H], hidden = h_T

Sharding: batch 256 -> 32 per core; weights replicated; the sequential time
loop runs locally per core (no collectives).

Device-side design (per core, B=32, H=1024, I=256, T=512):
  - The hidden state lives in a transposed, 32x32-block-permuted SBUF
    layout hT[p, 32w + b] = h[b, Hidx(p, w)], Hidx(p, w) =
    256*(p//32) + 32*w + (p%32). This makes the per-step transposition an
    in-place VectorE 32x32 stream-transpose, with the matching row
    permutation folded into a host-side pre-permuted W_hh^T.
  - Each step accumulates bias + W_in*x_t + W_hh*h into one PSUM tile
    [128, 256] (packed y[32g+b, c] = (preact)[b, 256g+c]) via col-tiled
    (tile_position) bf16 matmuls: 4 concurrent 32-batch-column streams
    per 128-contraction pass, 11 passes per step (1 bias + 2 input + 8
    recurrent), PSUM accumulating in fp32.
  - Epilogue per 128-col half: relu(0.1*y) (VectorE half / ScalarE half),
    VectorE stream-transpose, fused leaky-blend scalar_tensor_tensor into
    bf16 (next matmul operand) and fp32 (accumulator state, DMA'd out).
  - Front-matter matmuls of step t+1 are emitted before the epilogue of
    step t so the TensorE stays busy during the epilogue chain.
Host side pre-permutes/transposes the weights and decodes the per-step
block-transposed outputs.
"""

from contextlib import ExitStack

import numpy as np

import concourse.bacc as bacc
import concourse.mybir as mybir
import concourse.tile as tile
from concourse.bass_utils import run_bass_kernel_spmd

F32 = mybir.dt.float32
BF16 = mybir.dt.bfloat16
ALPHA = 0.1
T, B, I, H = 512, 256, 256, 1024
NCORES = 8
BS = B // NCORES  # 32

_CACHE = {}


def _build(tc, out_ap, x_ap, whhP_ap, winT_ap, bias_ap, sel4_ap, ident_ap):
    nc = tc.nc
    MAX = mybir.AluOpType.max
    MULT = mybir.AluOpType.mult
    ADD = mybir.AluOpType.add

    ctx = ExitStack()
    const_pool = ctx.enter_context(tc.tile_pool(name="const", bufs=1))
    state_pool = ctx.enter_context(tc.tile_pool(name="state", bufs=3))
    x_pool = ctx.enter_context(tc.tile_pool(name="x", bufs=3))
    xt_pool = ctx.enter_context(tc.tile_pool(name="xt", bufs=2))
    r_pool = ctx.enter_context(tc.tile_pool(name="r", bufs=2))
    rt_pool = ctx.enter_context(tc.tile_pool(name="rt", bufs=3))
    ypsum_pool = ctx.enter_context(tc.tile_pool(name="ypsum", bufs=2, space="PSUM"))
    xtpsum_pool = ctx.enter_context(tc.tile_pool(name="xtpsum", bufs=2, space="PSUM"))

    # --- constants (DMA f32 staging, DVE cast to matmul dtypes) ---
    stage = const_pool.tile([128, 8192], F32, tag="stage")
    whh_sb = const_pool.tile([128, 8192], BF16, tag="whh")
    nc.sync.dma_start(stage[:], whhP_ap[:])
    nc.vector.tensor_copy(whh_sb[:], stage[:])
    win_stage = const_pool.tile([128, 2048], F32, tag="win_stage")
    win_sb = const_pool.tile([128, 2048], BF16, tag="win")
    for i in range(2):
        nc.sync.dma_start(win_stage[:, 1024 * i:1024 * (i + 1)],
                          winT_ap[128 * i:128 * (i + 1), :])
    nc.vector.tensor_copy(win_sb[:], win_stage[:])
    bias_stage = const_pool.tile([4, 256], F32, tag="bias_stage")
    bias_sb = const_pool.tile([4, 256], BF16, tag="bias")
    nc.sync.dma_start(bias_stage[:], bias_ap[:])
    nc.vector.tensor_copy(bias_sb[:], bias_stage[:])
    sel4_stage = const_pool.tile([4, 128], F32, tag="sel4_stage")
    sel4_sb = const_pool.tile([4, 128], BF16, tag="sel4")
    nc.sync.dma_start(sel4_stage[:], sel4_ap[:])
    nc.vector.tensor_copy(sel4_sb[:], sel4_stage[:])
    ident_sb = const_pool.tile([128, 128], F32, tag="ident")
    nc.sync.dma_start(ident_sb[:], ident_ap[:])
    zero_f32 = const_pool.tile([128, 256], F32, tag="zero_f32")
    nc.vector.memset(zero_f32[:], 0.0)

    sT = state_pool.tile([128, 256], F32, tag="sT")
    nc.vector.tensor_copy(sT[:], zero_f32[:])
    hT16 = state_pool.tile([128, 256], BF16, tag="hT16")
    nc.vector.tensor_copy(hT16[:], zero_f32[:])

    xt4_sb = None

    def stage_x(q):
        nonlocal xt4_sb
        x4 = x_pool.tile([128, 256], F32, tag="x4")
        nc.sync.dma_start(x4[:], x_ap[4 * q:4 * q + 4].rearrange(
            "t b i -> (t b) i"))
        xtp = xtpsum_pool.tile([128, 256], F32, tag="xtp")
        for m in range(2):
            nc.tensor.transpose(xtp[:, 128 * m:128 * (m + 1)],
                                x4[:, 128 * m:128 * (m + 1)], ident_sb[:])
        xt4_sb = xt_pool.tile([128, 256], BF16, tag="xt4")
        nc.vector.tensor_copy(xt4_sb[:], xtp[:])

    def front_matter(t):
        """bias + input-projection matmuls for step t into a fresh psum."""
        yp = ypsum_pool.tile([128, 256], F32, tag="ypsum")
        nc.tensor.matmul(yp[:], sel4_sb[:], bias_sb[:],
                         start=True, stop=False, skip_group_check=True)
        tr = t % 4
        for m in range(2):
            lhs = xt4_sb[:, 128 * m + 32 * tr:128 * m + 32 * tr + 32]
            for g in range(4):
                nc.tensor.matmul(
                    yp[32 * g:32 * (g + 1), :], lhs,
                    win_sb[:, 1024 * m + 256 * g:1024 * m + 256 * (g + 1)],
                    start=False, stop=False, tile_position=(0, 32 * g),
                    skip_group_check=True)
        return yp

    stage_x(0)
    ypsum = front_matter(0)

    for t in range(T):
        # recurrent passes (need hT16 from previous step's blend)
        for w in range(8):
            for g in range(4):
                nc.tensor.matmul(
                    ypsum[32 * g:32 * (g + 1), :],
                    hT16[:, 32 * w:32 * (w + 1)],
                    whh_sb[:, 1024 * w + 256 * g:1024 * w + 256 * (g + 1)],
                    start=False, stop=False, tile_position=(0, 32 * g),
                    skip_group_check=True)

        # front matter of step t+1 keeps the PE busy during the epilogue
        ypsum_next = None
        if t + 1 < T:
            if (t + 1) % 4 == 0:
                stage_x((t + 1) // 4)
            ypsum_next = front_matter(t + 1)
            # low-power pacing: standalone weight load (1.2GHz, no MACs)
            # delays the next y-burst in-order without array power draw
            for _d in range(6):
                nc.tensor.ldweights(whh_sb[:, 128 * _d:128 * (_d + 1)])

        # epilogue, pipelined in 128-col halves:
        #   half0: relu on DVE -> stream-transpose -> blend(bf16)
        #   half1: relu on ACT -> stream-transpose -> blend(bf16)
        rt_sb = rt_pool.tile([128, 256], F32, tag="rt")
        hT16_new = state_pool.tile([128, 256], BF16, tag="hT16")
        hT_new = state_pool.tile([128, 256], F32, tag="hT")
        sT_new = state_pool.tile([128, 256], F32, tag="sT")
        h0, h1 = slice(0, 128), slice(128, 256)
        # per half: psum -> in-place 32x32 transpose -> h16 = max(rT,0) + s
        nc.vector.transpose(rt_sb[:, h0], ypsum[:, h0])
        nc.vector.scalar_tensor_tensor(
            hT16_new[:, h0], rt_sb[:, h0], 0.0, sT[:, h0], MAX, ADD)
        nc.vector.transpose(rt_sb[:, h1], ypsum[:, h1])
        nc.vector.scalar_tensor_tensor(
            hT16_new[:, h1], rt_sb[:, h1], 0.0, sT[:, h1], MAX, ADD)
        # off-chain: full-precision h, next-step leak state, output store
        nc.vector.scalar_tensor_tensor(
            hT_new[:], rt_sb[:], 0.0, sT[:], MAX, ADD)
        nc.scalar.activation(sT_new[:], hT_new[:],
                             mybir.ActivationFunctionType.Copy, scale=1.0 - ALPHA)
        nc.sync.dma_start(out_ap[t], hT_new[:])

        sT, hT16, ypsum = sT_new, hT16_new, ypsum_next

    ctx.close()


def _get_compiled():
    if "nc" in _CACHE:
        return _CACHE["nc"]
    nc = bacc.Bacc("TRN2", target_bir_lowering=False, debug=False,
                   num_devices=NCORES)
    aps = {}
    for name, shape in [("x", [T, BS, I]), ("whhP", [128, 8192]),
                        ("winT", [I, H]), ("bias", [4, 256]),
                        ("sel4", [4, 128]), ("ident", [128, 128])]:
        aps[name] = nc.dram_tensor(name, shape, F32, kind="ExternalInput").ap()
    out_ap = nc.dram_tensor("out", [T, 128, 256], F32,
                            kind="ExternalOutput").ap()
    with tile.TileContext(nc) as tc:
        _build(tc, out_ap, aps["x"], aps["whhP"], aps["winT"], aps["bias"],
               aps["sel4"], aps["ident"])
    nc.compile()
    _CACHE["nc"] = nc
    return nc


def host_prep(x_full, W_in, b_in, W_hh, b_hh):
    """Build per-core in_maps arrays (shared constants + per-core x shard)."""
    WT = np.ascontiguousarray(np.asarray(W_hh, np.float32).T * ALPHA)  # [k, j]
    p = np.arange(128)
    Hpw = 256 * (p[:, None] // 32) + 32 * np.arange(8)[None, :] + (p[:, None] % 32)
    whhP = np.ascontiguousarray(WT[Hpw].reshape(128, 8192))
    winT = np.ascontiguousarray(np.asarray(W_in, np.float32).T * ALPHA)
    bias = np.ascontiguousarray(
        ALPHA * (np.asarray(b_in, np.float32) + np.asarray(b_hh, np.float32)).reshape(4, 256))
    sel4 = np.ascontiguousarray(np.repeat(np.eye(4, dtype=np.float32), 32, axis=1))
    ident = np.eye(128, dtype=np.float32)
    x_full = np.asarray(x_full, np.float32)
    
    in_maps = []
    for c in range(NCORES):
        in_maps.append({
            "x": np.ascontiguousarray(x_full[:, c * BS:(c + 1) * BS, :]),
            "whhP": whhP, "winT": winT, "bias": bias, "sel4": sel4,
            "ident": ident,
        })
    return in_maps


def decode_out(arr):
    """[T, 128, 256] block-transposed -> [T, 32, 1024]."""
    t = arr.shape[0]
    tmp = arr.reshape(t, 4, 32, 8, 32)  # [t, pg, pv, w, b]
    return np.ascontiguousarray(tmp.transpose(0, 4, 1, 3, 2)).reshape(t, 32, 1024)


def run_on_hw(x, W_in, b_in, W_hh, b_hh, trace=False, **spmd_kwargs):
    nc = _get_compiled()
    in_maps = host_prep(x, W_in, b_in, W_hh, b_hh)
    res = run_bass_kernel_spmd(nc, in_maps, list(range(NCORES)), trace=trace,
                               **spmd_kwargs)
    shards = [decode_out(res.results[c]["out"]) for c in range(NCORES)]
    output = np.concatenate(shards, axis=1)  # [T, 256, 1024]
    return output, res


def kernel(x, W_in, b_in, W_hh, b_hh):
    output, _ = run_on_hw(x, W_in, b_in, W_hh, b_hh, trace=False)
    hidden = output[-1].copy()
    return output, hidden
